# revision 10
# baseline (speedup 1.0000x reference)
"""DCL contrastive loss kernel for Trainium2 (8 NeuronCores, Bass/Tile).

Problem: u, v [8192, 256] f32.
  sim_uv = cos_sim(u, v) / T ; sim_uu = cos_sim(u, u) / T   (T = 0.07)
  loss = mean_i( -sim_uv[i,i] + logsumexp_j(off-diag of [sim_uv | sim_uu] row i) )

Strategy (data-parallel rows, per the sharding hint):
  Phase 1 (SPMD, 8 cores): each core normalizes its 1024-row shard of u and v
    (fp32 row norms with Newton-refined rsqrt), emits bf16 unit rows.
  Host: build per-core column-ROLLED full matrices (core c's own rows first),
    so every core's diagonal block lands at a static column offset -> one SPMD
    program for all cores.
  Phase 2 (SPMD, 8 cores): each core loads the rolled matrices transposed
    (DMA-xbar), computes its [1024 x 8192] slab of both similarity matrices in
    bf16 matmuls (PSUM fp32), and streams exp((cos-1)/T) + row-sum through the
    scalar engine's fused accumulate. Fixed logsumexp shift C = 1/T (cos <= 1)
    means no max pass. Diagonals are masked in PSUM before exp; the uv diagonal
    (positive pair) is extracted with a masked multiply-reduce.
    Per row: loss = log(negsum) - (d_uv - 1)/T.  Output [128, 8] per core.
  Host: mean over all 8192 rows.

The `repeat` build parameter unrolls the whole phase body N times inside one
NEFF -- used only for benchmarking device time (launch overhead cancels).
"""

import sys

for _p in ("/opt/trn_rl_repo",):
    if _p not in sys.path:
        sys.path.insert(0, _p)

from contextlib import ExitStack

import numpy as np

import concourse.bass as bass
import concourse.tile as tile
from concourse import bacc, mybir
from concourse.bass_utils import run_bass_kernel_spmd

NCORES = 8
B, D = 8192, 256
SH = B // NCORES      # 1024 rows per core
PB = 128              # partition block
MB = SH // PB         # 8 row blocks per core
TEMP = 0.07
C = float(1.0 / TEMP)
GROUP = 2048          # columns per exp/accumulate group (4 PSUM banks)
NG = B // GROUP       # 4 groups per matrix
CHUNK = 512           # matmul moving free dim (1 PSUM bank)
NQ = GROUP // CHUNK   # 4 chunks per group
KD = D // PB          # 2 contraction halves

F32 = mybir.dt.float32
BF16 = mybir.dt.bfloat16
ALU = mybir.AluOpType
ACT = mybir.ActivationFunctionType

_PROGRAMS = {}


def _build_phase1(repeat=1):
    """Normalize shard rows: us, vs [1024, 256] f32 -> un, vn [1024, 256] bf16."""
    nc = bacc.Bacc("TRN2", target_bir_lowering=False, debug=False)
    us = nc.dram_tensor("us", [SH, D], F32, kind="ExternalInput")
    vs = nc.dram_tensor("vs", [SH, D], F32, kind="ExternalInput")
    un = nc.dram_tensor("un", [SH, D], BF16, kind="ExternalOutput")
    vn = nc.dram_tensor("vn", [SH, D], BF16, kind="ExternalOutput")

    with tile.TileContext(nc) as tc, ExitStack() as ctx:
        pool = ctx.enter_context(tc.tile_pool(name="main", bufs=2))
        sp = ctx.enter_context(tc.tile_pool(name="small", bufs=4))

        def body():
            for mi, (src, dst) in enumerate(((us, un), (vs, vn))):
                # one batched DMA per shard: [1024, 256] -> [128, 8, 256]
                x = pool.tile([PB, MB, D], F32, tag=f"x{mi}")
                nc.sync.dma_start(x[:], src[:].rearrange("(t p) d -> p t d", p=PB))
                # squared row norms for all 8 blocks -> ss_all [128, 8]
                ss_all = sp.tile([PB, MB], F32, tag=f"ss{mi}")
                for t in range(MB):
                    sq = pool.tile([PB, D], F32, tag=f"sq{mi}")
                    nc.vector.tensor_mul(sq[:], x[:, t, :], x[:, t, :])
                    nc.vector.reduce_sum(ss_all[:, t:t + 1], sq[:],
                                         axis=mybir.AxisListType.X)
                # r = 1/sqrt(ss), one batched Newton chain on [128, 8]
                nrm = sp.tile([PB, MB], F32, tag=f"nrm{mi}")
                nc.scalar.activation(nrm[:], ss_all[:], ACT.Sqrt)
                r = sp.tile([PB, MB], F32, tag=f"r0{mi}")
                nc.vector.reciprocal(r[:], nrm[:])
                for it in range(2):
                    t1 = sp.tile([PB, MB], F32, tag=f"t1_{it}{mi}")
                    nc.vector.tensor_mul(t1[:], r[:], r[:])
                    nc.vector.tensor_mul(t1[:], t1[:], ss_all[:])
                    nc.vector.tensor_scalar(t1[:], t1[:], -0.5, 1.5,
                                            ALU.mult, ALU.add)
                    rn = sp.tile([PB, MB], F32, tag=f"r{it + 1}{mi}")
                    nc.vector.tensor_mul(rn[:], r[:], t1[:])
                    r = rn
                y = pool.tile([PB, MB, D], BF16, tag=f"y{mi}")
                for t in range(MB):
                    nc.vector.tensor_scalar_mul(y[:, t, :], x[:, t, :],
                                                r[:, t:t + 1])
                nc.sync.dma_start(dst[:].rearrange("(t p) d -> p t d", p=PB), y[:])

        for _rep in range(repeat):
            body()
    nc.compile()
    return nc


def _build_phase2(repeat=1):
    """Similarity slab + streamed masked logsumexp.

    Inputs (per core, column-rolled so own rows are columns [0, 1024)):
      ur, vr [8192, 256] bf16; idmask/idneg [128, 128] f32 (eye, -300*eye).
    Output: loss [128, 8] f32; loss[p, m] is the per-row loss of local row
      m*128 + p.
    """
    nc = bacc.Bacc("TRN2", target_bir_lowering=False, debug=False)
    ur = nc.dram_tensor("ur", [B, D], BF16, kind="ExternalInput")
    vr = nc.dram_tensor("vr", [B, D], BF16, kind="ExternalInput")
    idmask = nc.dram_tensor("idmask", [PB, PB], F32, kind="ExternalInput")
    idneg = nc.dram_tensor("idneg", [PB, PB], F32, kind="ExternalInput")
    loss = nc.dram_tensor("loss", [PB, MB], F32, kind="ExternalOutput")

    with tile.TileContext(nc) as tc, ExitStack() as ctx:
        consts = ctx.enter_context(tc.tile_pool(name="consts", bufs=1))
        big = ctx.enter_context(tc.tile_pool(name="big", bufs=1))
        esc = ctx.enter_context(tc.tile_pool(name="esc", bufs=2))
        gs = ctx.enter_context(tc.tile_pool(name="gs", bufs=2))
        sm = ctx.enter_context(tc.tile_pool(name="sm", bufs=4))
        psum = ctx.enter_context(
            tc.tile_pool(name="psum", bufs=2, space=bass.MemorySpace.PSUM)
        )

        # constants
        idm = consts.tile([PB, PB], F32, tag="idm")
        nc.sync.dma_start(idm[:], idmask[:])
        idn = consts.tile([PB, PB], F32, tag="idn")
        nc.sync.dma_start(idn[:], idneg[:])
        biasc = consts.tile([PB, 1], F32, tag="biasc")
        nc.gpsimd.memset(biasc[:], -C)

        def body():
            # transposed unit matrices, loaded in 2048-column segments so
            # matmuls can start as soon as the first segments land
            xT = {}
            for s in range(NG):
                for nm, src in (("u", ur), ("v", vr)):
                    for k in range(KD):
                        t = big.tile([PB, GROUP], BF16, tag=f"{nm}T{k}s{s}")
                        nc.sync.dma_start_transpose(
                            t[:],
                            src[s * GROUP:(s + 1) * GROUP, k * PB:(k + 1) * PB],
                        )
                        xT[(nm, k, s)] = t

            negsum_all = consts.tile([PB, MB], F32, tag="negsum_all")
            duv_all = consts.tile([PB, MB], F32, tag="duv_all")

            for m in range(MB):
                grpsum = gs.tile([PB, 2 * NG], F32, tag="grpsum")
                off = m * PB  # static diag offset in group 0 (rolled layout)
                for xi, nm in enumerate(("v", "u")):
                    for g in range(NG):
                        ps = psum.tile([PB, GROUP], F32, tag="ps")
                        # k outer: stationary operand stays loaded across the
                        # group (2 weight loads per 8 matmuls instead of 8)
                        for k in range(KD):
                            for q in range(NQ):
                                nc.tensor.matmul(
                                    ps[:, q * CHUNK:(q + 1) * CHUNK],
                                    xT[("u", k, 0)][:, m * PB:(m + 1) * PB],
                                    xT[(nm, k, g)][:, q * CHUNK:(q + 1) * CHUNK],
                                    start=(k == 0),
                                    stop=(k == KD - 1),
                                    skip_group_check=True,
                                )
                        if g == 0:
                            if nm == "v":
                                scr = sm.tile([PB, PB], F32, tag="scr")
                                nc.vector.tensor_mul(
                                    scr[:], ps[:, off:off + PB], idm[:]
                                )
                                nc.vector.reduce_sum(
                                    duv_all[:, m:m + 1], scr[:],
                                    axis=mybir.AxisListType.X,
                                )
                            nc.vector.tensor_add(
                                ps[:, off:off + PB], ps[:, off:off + PB], idn[:]
                            )
                        escr = esc.tile([PB, GROUP], BF16, tag="escr")
                        col = xi * NG + g
                        nc.scalar.activation(
                            escr[:], ps[:], ACT.Exp,
                            bias=biasc[:], scale=C,
                            accum_out=grpsum[:, col:col + 1],
                        )
                nc.vector.reduce_sum(
                    negsum_all[:, m:m + 1], grpsum[:], axis=mybir.AxisListType.X
                )

            # single Ln at the end: one Exp->Ln table-set switch per kernel
            lg_all = consts.tile([PB, MB], F32, tag="lg_all")
            nc.scalar.activation(lg_all[:], negsum_all[:], ACT.Ln)
            posr_all = consts.tile([PB, MB], F32, tag="posr_all")
            nc.vector.tensor_scalar(posr_all[:], duv_all[:], -C, C,
                                    ALU.mult, ALU.add)
            lossT = consts.tile([PB, MB], F32, tag="loss")
            nc.vector.tensor_add(lossT[:], lg_all[:], posr_all[:])
            nc.sync.dma_start(loss[:], lossT[:])

        for _rep in range(repeat):
            body()
    nc.compile()
    return nc


def _get_programs():
    if "p1" not in _PROGRAMS:
        _PROGRAMS["p1"] = _build_phase1()
        _PROGRAMS["p2"] = _build_phase2()
    return _PROGRAMS["p1"], _PROGRAMS["p2"]


def make_phase2_inputs(un, vn):
    """Per-core phase-2 input maps from the 8 normalized shards."""
    eye = np.eye(PB, dtype=np.float32)
    eyen = (-300.0 * eye).astype(np.float32)
    in2 = []
    for c in range(NCORES):
        in2.append({
            "ur": np.concatenate(un[c:] + un[:c], axis=0),
            "vr": np.concatenate(vn[c:] + vn[:c], axis=0),
            "idmask": eye,
            "idneg": eyen,
        })
    return in2


def run_phases(u, v):
    """Returns (loss_scalar, phase1_results, phase2_results)."""
    u = np.ascontiguousarray(np.asarray(u, dtype=np.float32))
    v = np.ascontiguousarray(np.asarray(v, dtype=np.float32))
    assert u.shape == (B, D) and v.shape == (B, D)
    p1, p2 = _get_programs()
    cores = list(range(NCORES))

    in1 = [
        {"us": u[c * SH:(c + 1) * SH], "vs": v[c * SH:(c + 1) * SH]}
        for c in cores
    ]
    r1 = run_bass_kernel_spmd(p1, in1, cores)
    un = [r1.results[c]["un"] for c in cores]
    vn = [r1.results[c]["vn"] for c in cores]

    in2 = make_phase2_inputs(un, vn)
    r2 = run_bass_kernel_spmd(p2, in2, cores)
    losses = np.stack([r2.results[c]["loss"] for c in cores])  # [8, 128, 8]
    total = np.asarray(losses, dtype=np.float64).mean()
    return np.float32(total), r1, r2


def kernel(u, v):
    out, _, _ = run_phases(u, v)
    return np.asarray(out, dtype=np.float32)


if __name__ == "__main__":
    rng = np.random.default_rng(0)
    u = rng.standard_normal((B, D), dtype=np.float32)
    v = rng.standard_normal((B, D), dtype=np.float32)
    print("loss:", kernel(u, v))


# revision 11
# speedup vs baseline: 1.4226x; 1.4226x over previous
"""DCL contrastive loss kernel for Trainium2 (8 NeuronCores, Bass/Tile).

Problem: u, v [8192, 256] f32.
  sim_uv = cos_sim(u, v) / T ; sim_uu = cos_sim(u, u) / T   (T = 0.07)
  loss = mean_i( -sim_uv[i,i] + logsumexp_j(off-diag of [sim_uv | sim_uu] row i) )

Strategy (data-parallel rows, per the sharding hint):
  Phase 1 (SPMD, 8 cores): each core normalizes its 1024-row shard of u and v
    (fp32 row norms with Newton-refined rsqrt), emits bf16 unit rows.
  Host: build per-core column-ROLLED full matrices (core c's own rows first),
    so every core's diagonal block lands at a static column offset -> one SPMD
    program for all cores.
  Phase 2 (SPMD, 8 cores): each core loads the rolled matrices transposed
    (DMA-xbar), computes its [1024 x 8192] slab of both similarity matrices in
    bf16 matmuls (PSUM fp32), and streams exp((cos-1)/T) + row-sum through the
    scalar engine's fused accumulate. Fixed logsumexp shift C = 1/T (cos <= 1)
    means no max pass. Diagonals are masked in PSUM before exp; the uv diagonal
    (positive pair) is extracted with a masked multiply-reduce.
    Per row: loss = log(negsum) - (d_uv - 1)/T.  Output [128, 8] per core.
  Host: mean over all 8192 rows.

The `repeat` build parameter unrolls the whole phase body N times inside one
NEFF -- used only for benchmarking device time (launch overhead cancels).
"""

import sys

for _p in ("/opt/trn_rl_repo",):
    if _p not in sys.path:
        sys.path.insert(0, _p)

from contextlib import ExitStack

import numpy as np

import concourse.bass as bass
import concourse.tile as tile
from concourse import bacc, mybir
from concourse.bass_utils import run_bass_kernel_spmd

NCORES = 8
B, D = 8192, 256
SH = B // NCORES      # 1024 rows per core
PB = 128              # partition block
MB = SH // PB         # 8 row blocks per core
TEMP = 0.07
C = float(1.0 / TEMP)
GROUP = 2048          # columns per exp/accumulate group (4 PSUM banks)
NG = B // GROUP       # 4 groups per matrix
CHUNK = 512           # matmul moving free dim (1 PSUM bank)
NQ = GROUP // CHUNK   # 4 chunks per group
KD = D // PB          # 2 contraction halves

F32 = mybir.dt.float32
BF16 = mybir.dt.bfloat16
ALU = mybir.AluOpType
ACT = mybir.ActivationFunctionType

_PROGRAMS = {}


def _build_phase1(repeat=1, loop_n=0):
    """Normalize shard rows: us, vs [1024, 256] f32 -> un, vn [1024, 256] bf16."""
    nc = bacc.Bacc("TRN2", target_bir_lowering=False, debug=False)
    us = nc.dram_tensor("us", [SH, D], F32, kind="ExternalInput")
    vs = nc.dram_tensor("vs", [SH, D], F32, kind="ExternalInput")
    un = nc.dram_tensor("un", [SH, D], BF16, kind="ExternalOutput")
    vn = nc.dram_tensor("vn", [SH, D], BF16, kind="ExternalOutput")

    with tile.TileContext(nc) as tc, ExitStack() as ctx:
        pool = ctx.enter_context(tc.tile_pool(name="main", bufs=2))
        sp = ctx.enter_context(tc.tile_pool(name="small", bufs=4))

        def body():
            for mi, (src, dst) in enumerate(((us, un), (vs, vn))):
                # one batched DMA per shard: [1024, 256] -> [128, 8, 256]
                x = pool.tile([PB, MB, D], F32, tag=f"x{mi}")
                nc.sync.dma_start(x[:], src[:].rearrange("(t p) d -> p t d", p=PB))
                # squared row norms for all 8 blocks -> ss_all [128, 8]
                ss_all = sp.tile([PB, MB], F32, tag=f"ss{mi}")
                for t in range(MB):
                    sq = pool.tile([PB, D], F32, tag=f"sq{mi}")
                    nc.vector.tensor_mul(sq[:], x[:, t, :], x[:, t, :])
                    nc.vector.reduce_sum(ss_all[:, t:t + 1], sq[:],
                                         axis=mybir.AxisListType.X)
                # r = 1/sqrt(ss), one batched Newton chain on [128, 8]
                nrm = sp.tile([PB, MB], F32, tag=f"nrm{mi}")
                nc.scalar.activation(nrm[:], ss_all[:], ACT.Sqrt)
                r = sp.tile([PB, MB], F32, tag=f"r0{mi}")
                nc.vector.reciprocal(r[:], nrm[:])
                for it in range(2):
                    t1 = sp.tile([PB, MB], F32, tag=f"t1_{it}{mi}")
                    nc.vector.tensor_mul(t1[:], r[:], r[:])
                    nc.vector.tensor_mul(t1[:], t1[:], ss_all[:])
                    nc.vector.tensor_scalar(t1[:], t1[:], -0.5, 1.5,
                                            ALU.mult, ALU.add)
                    rn = sp.tile([PB, MB], F32, tag=f"r{it + 1}{mi}")
                    nc.vector.tensor_mul(rn[:], r[:], t1[:])
                    r = rn
                y = pool.tile([PB, MB, D], BF16, tag=f"y{mi}")
                for t in range(MB):
                    nc.vector.tensor_scalar_mul(y[:, t, :], x[:, t, :],
                                                r[:, t:t + 1])
                nc.sync.dma_start(dst[:].rearrange("(t p) d -> p t d", p=PB), y[:])

        if loop_n:
            with tc.For_i(0, loop_n, 1):
                body()
        else:
            for _rep in range(repeat):
                body()
    nc.compile()
    return nc


def _build_phase2(repeat=1, loop_n=0):
    """Similarity slab + streamed masked logsumexp.

    Inputs (per core, column-rolled so own rows are columns [0, 1024)):
      ur, vr [8192, 256] bf16; idmask/idneg [128, 128] f32 (eye, -300*eye).
    Output: loss [128, 8] f32; loss[p, m] is the per-row loss of local row
      m*128 + p.
    """
    nc = bacc.Bacc("TRN2", target_bir_lowering=False, debug=False)
    ur = nc.dram_tensor("ur", [B, D], BF16, kind="ExternalInput")
    vr = nc.dram_tensor("vr", [B, D], BF16, kind="ExternalInput")
    idmask = nc.dram_tensor("idmask", [PB, PB], F32, kind="ExternalInput")
    idneg = nc.dram_tensor("idneg", [PB, PB], F32, kind="ExternalInput")
    loss = nc.dram_tensor("loss", [PB, MB], F32, kind="ExternalOutput")

    with tile.TileContext(nc) as tc, ExitStack() as ctx:
        consts = ctx.enter_context(tc.tile_pool(name="consts", bufs=1))
        big = ctx.enter_context(tc.tile_pool(name="big", bufs=1))
        esc = ctx.enter_context(tc.tile_pool(name="esc", bufs=2))
        gs = ctx.enter_context(tc.tile_pool(name="gs", bufs=2))
        sm = ctx.enter_context(tc.tile_pool(name="sm", bufs=4))
        psum = ctx.enter_context(
            tc.tile_pool(name="psum", bufs=2, space=bass.MemorySpace.PSUM)
        )

        # constants
        idm = consts.tile([PB, PB], F32, tag="idm")
        nc.sync.dma_start(idm[:], idmask[:])
        idn = consts.tile([PB, PB], F32, tag="idn")
        nc.sync.dma_start(idn[:], idneg[:])
        biasc = consts.tile([PB, 1], F32, tag="biasc")
        nc.gpsimd.memset(biasc[:], -C)

        def body():
            # transposed unit matrices, loaded in 2048-column segments so
            # matmuls can start as soon as the first segments land
            xT = {}
            for s in range(NG):
                for nm, src in (("u", ur), ("v", vr)):
                    for k in range(KD):
                        t = big.tile([PB, GROUP], BF16, tag=f"{nm}T{k}s{s}")
                        nc.sync.dma_start_transpose(
                            t[:],
                            src[s * GROUP:(s + 1) * GROUP, k * PB:(k + 1) * PB],
                        )
                        xT[(nm, k, s)] = t

            negsum_all = consts.tile([PB, MB], F32, tag="negsum_all")
            duv_all = consts.tile([PB, MB], F32, tag="duv_all")

            for m in range(MB):
                grpsum = gs.tile([PB, 2 * NG], F32, tag="grpsum")
                off = m * PB  # static diag offset in group 0 (rolled layout)
                for xi, nm in enumerate(("v", "u")):
                    for g in range(NG):
                        ps = psum.tile([PB, GROUP], F32, tag="ps")
                        # k outer: stationary operand stays loaded across the
                        # group (2 weight loads per 8 matmuls instead of 8)
                        for k in range(KD):
                            for q in range(NQ):
                                nc.tensor.matmul(
                                    ps[:, q * CHUNK:(q + 1) * CHUNK],
                                    xT[("u", k, 0)][:, m * PB:(m + 1) * PB],
                                    xT[(nm, k, g)][:, q * CHUNK:(q + 1) * CHUNK],
                                    start=(k == 0),
                                    stop=(k == KD - 1),
                                    skip_group_check=True,
                                )
                        if g == 0:
                            if nm == "v":
                                scr = sm.tile([PB, PB], F32, tag="scr")
                                nc.vector.tensor_mul(
                                    scr[:], ps[:, off:off + PB], idm[:]
                                )
                                nc.vector.reduce_sum(
                                    duv_all[:, m:m + 1], scr[:],
                                    axis=mybir.AxisListType.X,
                                )
                            nc.vector.tensor_add(
                                ps[:, off:off + PB], ps[:, off:off + PB], idn[:]
                            )
                        escr = esc.tile([PB, GROUP], BF16, tag="escr")
                        col = xi * NG + g
                        nc.scalar.activation(
                            escr[:], ps[:], ACT.Exp,
                            bias=biasc[:], scale=C,
                            accum_out=grpsum[:, col:col + 1],
                        )
                nc.vector.reduce_sum(
                    negsum_all[:, m:m + 1], grpsum[:], axis=mybir.AxisListType.X
                )

            # single Ln at the end: one Exp->Ln table-set switch per kernel
            lg_all = consts.tile([PB, MB], F32, tag="lg_all")
            nc.scalar.activation(lg_all[:], negsum_all[:], ACT.Ln)
            posr_all = consts.tile([PB, MB], F32, tag="posr_all")
            nc.vector.tensor_scalar(posr_all[:], duv_all[:], -C, C,
                                    ALU.mult, ALU.add)
            lossT = consts.tile([PB, MB], F32, tag="loss")
            nc.vector.tensor_add(lossT[:], lg_all[:], posr_all[:])
            nc.sync.dma_start(loss[:], lossT[:])

        if loop_n:
            with tc.For_i(0, loop_n, 1):
                body()
        else:
            for _rep in range(repeat):
                body()
    nc.compile()
    return nc


def _get_programs():
    if "p1" not in _PROGRAMS:
        _PROGRAMS["p1"] = _build_phase1()
        _PROGRAMS["p2"] = _build_phase2()
    return _PROGRAMS["p1"], _PROGRAMS["p2"]


def make_phase2_inputs(un, vn):
    """Per-core phase-2 input maps from the 8 normalized shards."""
    eye = np.eye(PB, dtype=np.float32)
    eyen = (-300.0 * eye).astype(np.float32)
    in2 = []
    for c in range(NCORES):
        in2.append({
            "ur": np.concatenate(un[c:] + un[:c], axis=0),
            "vr": np.concatenate(vn[c:] + vn[:c], axis=0),
            "idmask": eye,
            "idneg": eyen,
        })
    return in2


def run_phases(u, v):
    """Returns (loss_scalar, phase1_results, phase2_results)."""
    u = np.ascontiguousarray(np.asarray(u, dtype=np.float32))
    v = np.ascontiguousarray(np.asarray(v, dtype=np.float32))
    assert u.shape == (B, D) and v.shape == (B, D)
    p1, p2 = _get_programs()
    cores = list(range(NCORES))

    in1 = [
        {"us": u[c * SH:(c + 1) * SH], "vs": v[c * SH:(c + 1) * SH]}
        for c in cores
    ]
    r1 = run_bass_kernel_spmd(p1, in1, cores)
    un = [r1.results[c]["un"] for c in cores]
    vn = [r1.results[c]["vn"] for c in cores]

    in2 = make_phase2_inputs(un, vn)
    r2 = run_bass_kernel_spmd(p2, in2, cores)
    losses = np.stack([r2.results[c]["loss"] for c in cores])  # [8, 128, 8]
    total = np.asarray(losses, dtype=np.float64).mean()
    return np.float32(total), r1, r2


def kernel(u, v):
    out, _, _ = run_phases(u, v)
    return np.asarray(out, dtype=np.float32)


if __name__ == "__main__":
    rng = np.random.default_rng(0)
    u = rng.standard_normal((B, D), dtype=np.float32)
    v = rng.standard_normal((B, D), dtype=np.float32)
    print("loss:", kernel(u, v))


# revision 14
# speedup vs baseline: 1.5945x; 1.1208x over previous
"""DCL contrastive loss kernel for Trainium2 (8 NeuronCores, Bass/Tile).

Problem: u, v [8192, 256] f32.
  sim_uv = cos_sim(u, v) / T ; sim_uu = cos_sim(u, u) / T   (T = 0.07)
  loss = mean_i( -sim_uv[i,i] + logsumexp_j(off-diag of [sim_uv | sim_uu] row i) )

Strategy (data-parallel rows, per the sharding hint):
  Phase 1 (SPMD, 8 cores): each core normalizes its 1024-row shard of u and v
    (fp32 row norms with Newton-refined rsqrt), emits bf16 unit rows.
  Host: build per-core column-ROLLED full matrices (core c's own rows first),
    so every core's diagonal block lands at a static column offset -> one SPMD
    program for all cores.
  Phase 2 (SPMD, 8 cores): each core loads the rolled matrices transposed
    (DMA-xbar), computes its [1024 x 8192] slab of both similarity matrices in
    bf16 matmuls (PSUM fp32), and streams exp((cos-1)/T) + row-sum through the
    scalar engine's fused accumulate. Fixed logsumexp shift C = 1/T (cos <= 1)
    means no max pass. Diagonals are masked in PSUM before exp; the uv diagonal
    (positive pair) is extracted with a masked multiply-reduce.
    Per row: loss = log(negsum) - (d_uv - 1)/T.  Output [128, 8] per core.
  Host: mean over all 8192 rows.

The `repeat` build parameter unrolls the whole phase body N times inside one
NEFF -- used only for benchmarking device time (launch overhead cancels).
"""

import sys

for _p in ("/opt/trn_rl_repo",):
    if _p not in sys.path:
        sys.path.insert(0, _p)

from contextlib import ExitStack

import numpy as np

import concourse.bass as bass
import concourse.tile as tile
from concourse import bacc, mybir
from concourse.bass_utils import run_bass_kernel_spmd

NCORES = 8
B, D = 8192, 256
SH = B // NCORES      # 1024 rows per core
PB = 128              # partition block
MB = SH // PB         # 8 row blocks per core
TEMP = 0.07
C = float(1.0 / TEMP)
GROUP = 2048          # columns per exp/accumulate group (4 PSUM banks)
NG = B // GROUP       # 4 groups per matrix
CHUNK = 512           # matmul moving free dim (1 PSUM bank)
NQ = GROUP // CHUNK   # 4 chunks per group
KD = D // PB          # 2 contraction halves

F32 = mybir.dt.float32
BF16 = mybir.dt.bfloat16
ALU = mybir.AluOpType
ACT = mybir.ActivationFunctionType

_PROGRAMS = {}


def _build_phase1(repeat=1, loop_n=0):
    """Normalize shard rows: us, vs [1024, 256] f32 -> un, vn [1024, 256] bf16."""
    nc = bacc.Bacc("TRN2", target_bir_lowering=False, debug=False)
    us = nc.dram_tensor("us", [SH, D], F32, kind="ExternalInput")
    vs = nc.dram_tensor("vs", [SH, D], F32, kind="ExternalInput")
    un = nc.dram_tensor("un", [SH, D], BF16, kind="ExternalOutput")
    vn = nc.dram_tensor("vn", [SH, D], BF16, kind="ExternalOutput")

    with tile.TileContext(nc) as tc, ExitStack() as ctx:
        pool = ctx.enter_context(tc.tile_pool(name="main", bufs=2))
        sp = ctx.enter_context(tc.tile_pool(name="small", bufs=4))

        def body():
            for mi, (src, dst) in enumerate(((us, un), (vs, vn))):
                # one batched DMA per shard: [1024, 256] -> [128, 8, 256]
                # u on the sync queue, v on the scalar queue (both HWDGE)
                dq = nc.sync if mi == 0 else nc.scalar
                x = pool.tile([PB, MB, D], F32, tag=f"x{mi}")
                dq.dma_start(x[:], src[:].rearrange("(t p) d -> p t d", p=PB))
                # squared row norms for all 8 blocks -> ss_all [128, 8]
                sq = pool.tile([PB, MB, D], F32, tag=f"sq{mi}")
                nc.vector.tensor_mul(sq[:], x[:], x[:])
                ss_all = sp.tile([PB, MB], F32, tag=f"ss{mi}")
                nc.vector.reduce_sum(ss_all[:], sq[:],
                                     axis=mybir.AxisListType.X)
                # r = 1/sqrt(ss), one batched Newton chain on [128, 8]
                nrm = sp.tile([PB, MB], F32, tag=f"nrm{mi}")
                nc.scalar.activation(nrm[:], ss_all[:], ACT.Sqrt)
                r = sp.tile([PB, MB], F32, tag=f"r0{mi}")
                nc.vector.reciprocal(r[:], nrm[:])
                for it in range(2):
                    t1 = sp.tile([PB, MB], F32, tag=f"t1_{it}{mi}")
                    nc.vector.tensor_mul(t1[:], r[:], r[:])
                    nc.vector.tensor_mul(t1[:], t1[:], ss_all[:])
                    nc.vector.tensor_scalar(t1[:], t1[:], -0.5, 1.5,
                                            ALU.mult, ALU.add)
                    rn = sp.tile([PB, MB], F32, tag=f"r{it + 1}{mi}")
                    nc.vector.tensor_mul(rn[:], r[:], t1[:])
                    r = rn
                y = pool.tile([PB, MB, D], BF16, tag=f"y{mi}")
                for t in range(MB):
                    nc.vector.tensor_scalar_mul(y[:, t, :], x[:, t, :],
                                                r[:, t:t + 1])
                dq.dma_start(dst[:].rearrange("(t p) d -> p t d", p=PB), y[:])

        if loop_n:
            with tc.For_i(0, loop_n, 1):
                body()
        else:
            for _rep in range(repeat):
                body()
    nc.compile()
    return nc


def _build_phase2(repeat=1, loop_n=0):
    """Similarity slab + streamed masked logsumexp.

    Inputs (per core, column-rolled so own rows are columns [0, 1024)):
      ur, vr [8192, 256] bf16; idmask/idneg [128, 128] f32 (eye, -300*eye).
    Output: loss [128, 8] f32; loss[p, m] is the per-row loss of local row
      m*128 + p.
    """
    nc = bacc.Bacc("TRN2", target_bir_lowering=False, debug=False)
    ur = nc.dram_tensor("ur", [B, D], BF16, kind="ExternalInput")
    vr = nc.dram_tensor("vr", [B, D], BF16, kind="ExternalInput")
    idmask = nc.dram_tensor("idmask", [PB, PB], F32, kind="ExternalInput")
    idneg = nc.dram_tensor("idneg", [PB, PB], F32, kind="ExternalInput")
    loss = nc.dram_tensor("loss", [PB, MB], F32, kind="ExternalOutput")

    with tile.TileContext(nc) as tc, ExitStack() as ctx:
        consts = ctx.enter_context(tc.tile_pool(name="consts", bufs=1))
        big = ctx.enter_context(tc.tile_pool(name="big", bufs=1))
        esc = ctx.enter_context(tc.tile_pool(name="esc", bufs=2))
        gs = ctx.enter_context(tc.tile_pool(name="gs", bufs=2))
        sm = ctx.enter_context(tc.tile_pool(name="sm", bufs=4))
        psum = ctx.enter_context(
            tc.tile_pool(name="psum", bufs=2, space=bass.MemorySpace.PSUM)
        )

        # constants
        idm = consts.tile([PB, PB], F32, tag="idm")
        nc.sync.dma_start(idm[:], idmask[:])
        idn = consts.tile([PB, PB], F32, tag="idn")
        nc.sync.dma_start(idn[:], idneg[:])
        biasc = consts.tile([PB, 1], F32, tag="biasc")
        nc.gpsimd.memset(biasc[:], -C)

        def body():
            # transposed unit matrices, loaded in 2048-column segments so
            # matmuls can start as soon as the first segments land
            xT = {}
            for s in range(NG):
                for nm, src in (("u", ur), ("v", vr)):
                    for k in range(KD):
                        t = big.tile([PB, GROUP], BF16, tag=f"{nm}T{k}s{s}")
                        nc.sync.dma_start_transpose(
                            t[:],
                            src[s * GROUP:(s + 1) * GROUP, k * PB:(k + 1) * PB],
                        )
                        xT[(nm, k, s)] = t

            negsum_all = consts.tile([PB, MB], F32, tag="negsum_all")
            duv_all = consts.tile([PB, MB], F32, tag="duv_all")
            duu_all = consts.tile([PB, MB], F32, tag="duu_all")

            for m in range(MB):
                grpsum = gs.tile([PB, 2 * NG], F32, tag="grpsum")
                off = m * PB  # static diag offset in group 0 (rolled layout)
                for xi, nm in enumerate(("v", "u")):
                    for g in range(NG):
                        ps = psum.tile([PB, GROUP], F32, tag="ps")
                        # k outer: stationary operand stays loaded across the
                        # group (2 weight loads per 8 matmuls instead of 8)
                        for k in range(KD):
                            for q in range(NQ):
                                nc.tensor.matmul(
                                    ps[:, q * CHUNK:(q + 1) * CHUNK],
                                    xT[("u", k, 0)][:, m * PB:(m + 1) * PB],
                                    xT[(nm, k, g)][:, q * CHUNK:(q + 1) * CHUNK],
                                    start=(k == 0),
                                    stop=(k == KD - 1),
                                    skip_group_check=True,
                                )
                        if g == 0:
                            # read-only diagonal extraction (does not block
                            # the exp pass; its contribution is subtracted
                            # from the accumulated sums at the end)
                            dall = duv_all if nm == "v" else duu_all
                            scr = sm.tile([PB, PB], F32, tag="scr")
                            nc.vector.tensor_mul(
                                scr[:], ps[:, off:off + PB], idm[:]
                            )
                            nc.vector.reduce_sum(
                                dall[:, m:m + 1], scr[:],
                                axis=mybir.AxisListType.X,
                            )
                        escr = esc.tile([PB, GROUP], BF16, tag="escr")
                        col = xi * NG + g
                        nc.scalar.activation(
                            escr[:], ps[:], ACT.Exp,
                            bias=biasc[:], scale=C,
                            accum_out=grpsum[:, col:col + 1],
                        )
                nc.vector.reduce_sum(
                    negsum_all[:, m:m + 1], grpsum[:], axis=mybir.AxisListType.X
                )

            # tail (one Exp->Ln table-set switch per kernel):
            #   negsum -= exp((d_uv-1)C) + exp((d_uu-1)C)   [diag removal]
            #   loss = log(negsum) - (d_uv - 1)*C
            tuv = consts.tile([PB, MB], F32, tag="tuv")
            nc.scalar.activation(tuv[:], duv_all[:], ACT.Exp,
                                 bias=biasc[:], scale=C)
            tuu = consts.tile([PB, MB], F32, tag="tuu")
            nc.scalar.activation(tuu[:], duu_all[:], ACT.Exp,
                                 bias=biasc[:], scale=C)
            nc.vector.tensor_sub(negsum_all[:], negsum_all[:], tuv[:])
            nc.vector.tensor_sub(negsum_all[:], negsum_all[:], tuu[:])
            lg_all = consts.tile([PB, MB], F32, tag="lg_all")
            nc.scalar.activation(lg_all[:], negsum_all[:], ACT.Ln)
            posr_all = consts.tile([PB, MB], F32, tag="posr_all")
            nc.vector.tensor_scalar(posr_all[:], duv_all[:], -C, C,
                                    ALU.mult, ALU.add)
            lossT = consts.tile([PB, MB], F32, tag="loss")
            nc.vector.tensor_add(lossT[:], lg_all[:], posr_all[:])
            nc.sync.dma_start(loss[:], lossT[:])

        if loop_n:
            with tc.For_i(0, loop_n, 1):
                body()
        else:
            for _rep in range(repeat):
                body()
    nc.compile()
    return nc


def _get_programs():
    if "p1" not in _PROGRAMS:
        _PROGRAMS["p1"] = _build_phase1()
        _PROGRAMS["p2"] = _build_phase2()
    return _PROGRAMS["p1"], _PROGRAMS["p2"]


def make_phase2_inputs(un, vn):
    """Per-core phase-2 input maps from the 8 normalized shards."""
    eye = np.eye(PB, dtype=np.float32)
    eyen = (-300.0 * eye).astype(np.float32)
    in2 = []
    for c in range(NCORES):
        in2.append({
            "ur": np.concatenate(un[c:] + un[:c], axis=0),
            "vr": np.concatenate(vn[c:] + vn[:c], axis=0),
            "idmask": eye,
            "idneg": eyen,
        })
    return in2


def run_phases(u, v):
    """Returns (loss_scalar, phase1_results, phase2_results)."""
    u = np.ascontiguousarray(np.asarray(u, dtype=np.float32))
    v = np.ascontiguousarray(np.asarray(v, dtype=np.float32))
    assert u.shape == (B, D) and v.shape == (B, D)
    p1, p2 = _get_programs()
    cores = list(range(NCORES))

    in1 = [
        {"us": u[c * SH:(c + 1) * SH], "vs": v[c * SH:(c + 1) * SH]}
        for c in cores
    ]
    r1 = run_bass_kernel_spmd(p1, in1, cores)
    un = [r1.results[c]["un"] for c in cores]
    vn = [r1.results[c]["vn"] for c in cores]

    in2 = make_phase2_inputs(un, vn)
    r2 = run_bass_kernel_spmd(p2, in2, cores)
    losses = np.stack([r2.results[c]["loss"] for c in cores])  # [8, 128, 8]
    total = np.asarray(losses, dtype=np.float64).mean()
    return np.float32(total), r1, r2


def kernel(u, v):
    out, _, _ = run_phases(u, v)
    return np.asarray(out, dtype=np.float32)


if __name__ == "__main__":
    rng = np.random.default_rng(0)
    u = rng.standard_normal((B, D), dtype=np.float32)
    v = rng.standard_normal((B, D), dtype=np.float32)
    print("loss:", kernel(u, v))


# revision 18
# speedup vs baseline: 1.6108x; 1.0102x over previous
"""DCL contrastive loss kernel for Trainium2 (8 NeuronCores, Bass/Tile).

Problem: u, v [8192, 256] f32.
  sim_uv = cos_sim(u, v) / T ; sim_uu = cos_sim(u, u) / T   (T = 0.07)
  loss = mean_i( -sim_uv[i,i] + logsumexp_j(off-diag of [sim_uv | sim_uu] row i) )

Strategy (data-parallel rows, per the sharding hint):
  Phase 1 (SPMD, 8 cores): each core normalizes its 1024-row shard of u and v
    (fp32 row norms with Newton-refined rsqrt), emits bf16 unit rows.
  Host: build per-core column-ROLLED full matrices (core c's own rows first),
    so every core's diagonal block lands at a static column offset -> one SPMD
    program for all cores.
  Phase 2 (SPMD, 8 cores): each core loads the rolled matrices transposed
    (DMA-xbar), computes its [1024 x 8192] slab of both similarity matrices in
    bf16 matmuls (PSUM fp32), and streams exp((cos-1)/T) + row-sum through the
    scalar engine's fused accumulate. Fixed logsumexp shift C = 1/T (cos <= 1)
    means no max pass. Diagonals are masked in PSUM before exp; the uv diagonal
    (positive pair) is extracted with a masked multiply-reduce.
    Per row: loss = log(negsum) - (d_uv - 1)/T.  Output [128, 8] per core.
  Host: mean over all 8192 rows.

The `repeat` build parameter unrolls the whole phase body N times inside one
NEFF -- used only for benchmarking device time (launch overhead cancels).
"""

import sys

for _p in ("/opt/trn_rl_repo",):
    if _p not in sys.path:
        sys.path.insert(0, _p)

from contextlib import ExitStack

import numpy as np

import concourse.bass as bass
import concourse.tile as tile
from concourse import bacc, mybir
from concourse.bass_utils import run_bass_kernel_spmd

NCORES = 8
B, D = 8192, 256
SH = B // NCORES      # 1024 rows per core
PB = 128              # partition block
MB = SH // PB         # 8 row blocks per core
TEMP = 0.07
C = float(1.0 / TEMP)
GROUP = 2048          # columns per exp/accumulate group (4 PSUM banks)
NG = B // GROUP       # 4 groups per matrix
CHUNK = 512           # matmul moving free dim (1 PSUM bank)
NQ = GROUP // CHUNK   # 4 chunks per group
KD = D // PB          # 2 contraction halves

F32 = mybir.dt.float32
BF16 = mybir.dt.bfloat16
ALU = mybir.AluOpType
ACT = mybir.ActivationFunctionType

_PROGRAMS = {}


def _build_phase1(repeat=1, loop_n=0):
    """Normalize shard rows and emit them TRANSPOSED:
    us, vs [1024, 256] f32 -> unT, vnT [256, 1024] bf16 (unit rows as columns).
    """
    nc = bacc.Bacc("TRN2", target_bir_lowering=False, debug=False)
    us = nc.dram_tensor("us", [SH, D], F32, kind="ExternalInput")
    vs = nc.dram_tensor("vs", [SH, D], F32, kind="ExternalInput")
    idbf = nc.dram_tensor("idbf", [PB, PB], BF16, kind="ExternalInput")
    unT = nc.dram_tensor("unT", [D, SH], BF16, kind="ExternalOutput")
    vnT = nc.dram_tensor("vnT", [D, SH], BF16, kind="ExternalOutput")

    with tile.TileContext(nc) as tc, ExitStack() as ctx:
        pool = ctx.enter_context(tc.tile_pool(name="main", bufs=2))
        sp = ctx.enter_context(tc.tile_pool(name="small", bufs=4))
        consts = ctx.enter_context(tc.tile_pool(name="consts", bufs=1))
        psum = ctx.enter_context(
            tc.tile_pool(name="psum", bufs=4, space=bass.MemorySpace.PSUM)
        )
        idt = consts.tile([PB, PB], BF16, tag="idt")
        nc.sync.dma_start(idt[:], idbf[:])

        def body():
            for mi, (src, dst) in enumerate(((us, unT), (vs, vnT))):
                # one batched DMA per shard: [1024, 256] -> [128, 8, 256]
                # u on the sync queue, v on the scalar queue (both HWDGE)
                dq = nc.sync if mi == 0 else nc.scalar
                x = pool.tile([PB, MB, D], F32, tag=f"x{mi}")
                dq.dma_start(x[:], src[:].rearrange("(t p) d -> p t d", p=PB))
                # squared row norms for all 8 blocks -> ss_all [128, 8]
                sq = pool.tile([PB, MB, D], F32, tag=f"sq{mi}")
                nc.vector.tensor_mul(sq[:], x[:], x[:])
                ss_all = sp.tile([PB, MB], F32, tag=f"ss{mi}")
                nc.vector.reduce_sum(ss_all[:], sq[:],
                                     axis=mybir.AxisListType.X)
                # r = 1/sqrt(ss), one batched Newton chain on [128, 8]
                nrm = sp.tile([PB, MB], F32, tag=f"nrm{mi}")
                nc.scalar.activation(nrm[:], ss_all[:], ACT.Sqrt)
                r = sp.tile([PB, MB], F32, tag=f"r0{mi}")
                nc.vector.reciprocal(r[:], nrm[:])
                for it in range(2):
                    t1 = sp.tile([PB, MB], F32, tag=f"t1_{it}{mi}")
                    nc.vector.tensor_mul(t1[:], r[:], r[:])
                    nc.vector.tensor_mul(t1[:], t1[:], ss_all[:])
                    nc.vector.tensor_scalar(t1[:], t1[:], -0.5, 1.5,
                                            ALU.mult, ALU.add)
                    rn = sp.tile([PB, MB], F32, tag=f"r{it + 1}{mi}")
                    nc.vector.tensor_mul(rn[:], r[:], t1[:])
                    r = rn
                y = pool.tile([PB, MB, D], BF16, tag=f"y{mi}")
                for t in range(MB):
                    nc.vector.tensor_scalar_mul(y[:, t, :], x[:, t, :],
                                                r[:, t:t + 1])
                # transpose 128x128 blocks on the (otherwise idle) PE
                yT = pool.tile([PB, KD, SH], BF16, tag=f"yT{mi}")
                for k in range(KD):
                    for t in range(MB):
                        pst = psum.tile([PB, PB], BF16, tag=f"pst{mi}")
                        nc.tensor.transpose(
                            pst[:], y[:, t, k * PB:(k + 1) * PB], idt[:]
                        )
                        nc.vector.tensor_copy(
                            yT[:, k, t * PB:(t + 1) * PB], pst[:]
                        )
                dq.dma_start(dst[:].rearrange("(k dd) c -> dd k c", dd=PB),
                             yT[:])

        if loop_n:
            with tc.For_i(0, loop_n, 1):
                body()
        else:
            for _rep in range(repeat):
                body()
    nc.compile()
    return nc


def _build_phase2(repeat=1, loop_n=0):
    """Similarity slab + streamed masked logsumexp.

    Inputs (per core, column-rolled so own rows are columns [0, 1024)):
      ur, vr [8192, 256] bf16; idmask/idneg [128, 128] f32 (eye, -300*eye).
    Output: loss [128, 8] f32; loss[p, m] is the per-row loss of local row
      m*128 + p.
    """
    nc = bacc.Bacc("TRN2", target_bir_lowering=False, debug=False)
    urT = nc.dram_tensor("urT", [D, B], BF16, kind="ExternalInput")
    vrT = nc.dram_tensor("vrT", [D, B], BF16, kind="ExternalInput")
    idmask = nc.dram_tensor("idmask", [PB, PB], F32, kind="ExternalInput")
    loss = nc.dram_tensor("loss", [PB, MB], F32, kind="ExternalOutput")

    with tile.TileContext(nc) as tc, ExitStack() as ctx:
        consts = ctx.enter_context(tc.tile_pool(name="consts", bufs=1))
        big = ctx.enter_context(tc.tile_pool(name="big", bufs=1))
        esc = ctx.enter_context(tc.tile_pool(name="esc", bufs=2))
        gs = ctx.enter_context(tc.tile_pool(name="gs", bufs=2))
        sm = ctx.enter_context(tc.tile_pool(name="sm", bufs=4))
        psum = ctx.enter_context(
            tc.tile_pool(name="psum", bufs=2, space=bass.MemorySpace.PSUM)
        )

        # constants
        idm = consts.tile([PB, PB], F32, tag="idm")
        nc.sync.dma_start(idm[:], idmask[:])
        biasc = consts.tile([PB, 1], F32, tag="biasc")
        nc.gpsimd.memset(biasc[:], -C)
        # trigger the exp/ln ACT table load early (overlaps input DMA)
        actwarm = consts.tile([PB, 1], F32, tag="actwarm")
        nc.scalar.activation(actwarm[:], biasc[:], ACT.Exp,
                             bias=biasc[:], scale=C)

        def body():
            # transposed unit matrices in 2048-column segments, split across
            # both HWDGE queues; matmuls start when the first segments land
            xT = {}
            qi = 0
            for s in range(NG):
                for nm, src in (("u", urT), ("v", vrT)):
                    for k in range(KD):
                        t = big.tile([PB, GROUP], BF16, tag=f"{nm}T{k}s{s}")
                        eng = nc.sync if qi % 2 == 0 else nc.gpsimd
                        eng.dma_start(
                            t[:],
                            src[k * PB:(k + 1) * PB,
                                s * GROUP:(s + 1) * GROUP],
                        )
                        qi += 1
                        xT[(nm, k, s)] = t

            # warm the PE (HAM clock gate) with throwaway matmuls on the
            # identity tile while the input DMAs stream in
            wps = psum.tile([PB, GROUP], F32, tag="ps")
            for _w in range(16):
                nc.tensor.matmul(wps[:, 0:PB], idm[:], idm[:],
                                 start=True, stop=True, skip_group_check=True)

            negsum_all = consts.tile([PB, MB], F32, tag="negsum_all")
            duv_all = consts.tile([PB, MB], F32, tag="duv_all")
            duu_all = consts.tile([PB, MB], F32, tag="duu_all")

            for m in range(MB):
                grpsum = gs.tile([PB, 2 * NG], F32, tag="grpsum")
                off = m * PB  # static diag offset in group 0 (rolled layout)
                for xi, nm in enumerate(("v", "u")):
                    for g in range(NG):
                        ps = psum.tile([PB, GROUP], F32, tag="ps")
                        # k outer: stationary operand stays loaded across the
                        # group (2 weight loads per 8 matmuls instead of 8)
                        for k in range(KD):
                            for q in range(NQ):
                                nc.tensor.matmul(
                                    ps[:, q * CHUNK:(q + 1) * CHUNK],
                                    xT[("u", k, 0)][:, m * PB:(m + 1) * PB],
                                    xT[(nm, k, g)][:, q * CHUNK:(q + 1) * CHUNK],
                                    start=(k == 0),
                                    stop=(k == KD - 1),
                                    skip_group_check=True,
                                )
                        if g == 0:
                            # read-only diagonal extraction (does not block
                            # the exp pass; its contribution is subtracted
                            # from the accumulated sums at the end)
                            dall = duv_all if nm == "v" else duu_all
                            scr = sm.tile([PB, PB], F32, tag="scr")
                            nc.vector.tensor_mul(
                                scr[:], ps[:, off:off + PB], idm[:]
                            )
                            nc.vector.reduce_sum(
                                dall[:, m:m + 1], scr[:],
                                axis=mybir.AxisListType.X,
                            )
                        escr = esc.tile([PB, GROUP], BF16, tag="escr")
                        col = xi * NG + g
                        nc.scalar.activation(
                            escr[:], ps[:], ACT.Exp,
                            bias=biasc[:], scale=C,
                            accum_out=grpsum[:, col:col + 1],
                        )
                nc.vector.reduce_sum(
                    negsum_all[:, m:m + 1], grpsum[:], axis=mybir.AxisListType.X
                )

            # tail (one Exp->Ln table-set switch per kernel):
            #   negsum -= exp((d_uv-1)C) + exp((d_uu-1)C)   [diag removal]
            #   loss = log(negsum) - (d_uv - 1)*C
            tuv = consts.tile([PB, MB], F32, tag="tuv")
            nc.scalar.activation(tuv[:], duv_all[:], ACT.Exp,
                                 bias=biasc[:], scale=C)
            tuu = consts.tile([PB, MB], F32, tag="tuu")
            nc.scalar.activation(tuu[:], duu_all[:], ACT.Exp,
                                 bias=biasc[:], scale=C)
            nc.vector.tensor_sub(negsum_all[:], negsum_all[:], tuv[:])
            nc.vector.tensor_sub(negsum_all[:], negsum_all[:], tuu[:])
            lg_all = consts.tile([PB, MB], F32, tag="lg_all")
            nc.scalar.activation(lg_all[:], negsum_all[:], ACT.Ln)
            posr_all = consts.tile([PB, MB], F32, tag="posr_all")
            nc.vector.tensor_scalar(posr_all[:], duv_all[:], -C, C,
                                    ALU.mult, ALU.add)
            lossT = consts.tile([PB, MB], F32, tag="loss")
            nc.vector.tensor_add(lossT[:], lg_all[:], posr_all[:])
            nc.sync.dma_start(loss[:], lossT[:])

        if loop_n:
            with tc.For_i(0, loop_n, 1):
                body()
        else:
            for _rep in range(repeat):
                body()
    nc.compile()
    return nc


def _get_programs():
    if "p1" not in _PROGRAMS:
        _PROGRAMS["p1"] = _build_phase1()
        _PROGRAMS["p2"] = _build_phase2()
    return _PROGRAMS["p1"], _PROGRAMS["p2"]


def make_phase1_inputs(u, v):
    import ml_dtypes
    eye_bf = np.eye(PB, dtype=ml_dtypes.bfloat16)
    return [
        {"us": u[c * SH:(c + 1) * SH], "vs": v[c * SH:(c + 1) * SH],
         "idbf": eye_bf}
        for c in range(NCORES)
    ]


def make_phase2_inputs(unT, vnT):
    """Per-core phase-2 input maps from the 8 normalized transposed shards,
    column-rolled so each core's own rows come first."""
    eye = np.eye(PB, dtype=np.float32)
    in2 = []
    for c in range(NCORES):
        in2.append({
            "urT": np.concatenate(unT[c:] + unT[:c], axis=1),
            "vrT": np.concatenate(vnT[c:] + vnT[:c], axis=1),
            "idmask": eye,
        })
    return in2


def run_phases(u, v):
    """Returns (loss_scalar, phase1_results, phase2_results)."""
    u = np.ascontiguousarray(np.asarray(u, dtype=np.float32))
    v = np.ascontiguousarray(np.asarray(v, dtype=np.float32))
    assert u.shape == (B, D) and v.shape == (B, D)
    p1, p2 = _get_programs()
    cores = list(range(NCORES))

    in1 = make_phase1_inputs(u, v)
    r1 = run_bass_kernel_spmd(p1, in1, cores)
    unT = [r1.results[c]["unT"] for c in cores]
    vnT = [r1.results[c]["vnT"] for c in cores]

    in2 = make_phase2_inputs(unT, vnT)
    r2 = run_bass_kernel_spmd(p2, in2, cores)
    losses = np.stack([r2.results[c]["loss"] for c in cores])  # [8, 128, 8]
    total = np.asarray(losses, dtype=np.float64).mean()
    return np.float32(total), r1, r2


def kernel(u, v):
    out, _, _ = run_phases(u, v)
    return np.asarray(out, dtype=np.float32)


if __name__ == "__main__":
    rng = np.random.default_rng(0)
    u = rng.standard_normal((B, D), dtype=np.float32)
    v = rng.standard_normal((B, D), dtype=np.float32)
    print("loss:", kernel(u, v))


# revision 19
# speedup vs baseline: 1.6972x; 1.0536x over previous
"""DCL contrastive loss kernel for Trainium2 (8 NeuronCores, Bass/Tile).

Problem: u, v [8192, 256] f32.
  sim_uv = cos_sim(u, v) / T ; sim_uu = cos_sim(u, u) / T   (T = 0.07)
  loss = mean_i( -sim_uv[i,i] + logsumexp_j(off-diag of [sim_uv | sim_uu] row i) )

Strategy (data-parallel rows, per the sharding hint):
  Phase 1 (SPMD, 8 cores): each core normalizes its 1024-row shard of u and v
    (fp32 row norms with Newton-refined rsqrt), emits bf16 unit rows.
  Host: build per-core column-ROLLED full matrices (core c's own rows first),
    so every core's diagonal block lands at a static column offset -> one SPMD
    program for all cores.
  Phase 2 (SPMD, 8 cores): each core loads the rolled matrices transposed
    (DMA-xbar), computes its [1024 x 8192] slab of both similarity matrices in
    bf16 matmuls (PSUM fp32), and streams exp((cos-1)/T) + row-sum through the
    scalar engine's fused accumulate. Fixed logsumexp shift C = 1/T (cos <= 1)
    means no max pass. Diagonals are masked in PSUM before exp; the uv diagonal
    (positive pair) is extracted with a masked multiply-reduce.
    Per row: loss = log(negsum) - (d_uv - 1)/T.  Output [128, 8] per core.
  Host: mean over all 8192 rows.

The `repeat` build parameter unrolls the whole phase body N times inside one
NEFF -- used only for benchmarking device time (launch overhead cancels).
"""

import sys

for _p in ("/opt/trn_rl_repo",):
    if _p not in sys.path:
        sys.path.insert(0, _p)

from contextlib import ExitStack

import numpy as np

import concourse.bass as bass
import concourse.tile as tile
from concourse import bacc, mybir
from concourse.bass_utils import run_bass_kernel_spmd

NCORES = 8
B, D = 8192, 256
SH = B // NCORES      # 1024 rows per core
PB = 128              # partition block
MB = SH // PB         # 8 row blocks per core
TEMP = 0.07
C = float(1.0 / TEMP)
GROUP = 2048          # columns per exp/accumulate group (4 PSUM banks)
NG = B // GROUP       # 4 groups per matrix
CHUNK = 512           # matmul moving free dim (1 PSUM bank)
NQ = GROUP // CHUNK   # 4 chunks per group
KD = D // PB          # 2 contraction halves

F32 = mybir.dt.float32
BF16 = mybir.dt.bfloat16
ALU = mybir.AluOpType
ACT = mybir.ActivationFunctionType

_PROGRAMS = {}


def _build_phase1(repeat=1, loop_n=0):
    """Normalize shard rows and emit them TRANSPOSED:
    us, vs [1024, 256] f32 -> unT, vnT [256, 1024] bf16 (unit rows as columns).
    """
    nc = bacc.Bacc("TRN2", target_bir_lowering=False, debug=False)
    us = nc.dram_tensor("us", [SH, D], F32, kind="ExternalInput")
    vs = nc.dram_tensor("vs", [SH, D], F32, kind="ExternalInput")
    idbf = nc.dram_tensor("idbf", [PB, PB], BF16, kind="ExternalInput")
    unT = nc.dram_tensor("unT", [D, SH], BF16, kind="ExternalOutput")
    vnT = nc.dram_tensor("vnT", [D, SH], BF16, kind="ExternalOutput")

    with tile.TileContext(nc) as tc, ExitStack() as ctx:
        pool = ctx.enter_context(tc.tile_pool(name="main", bufs=2))
        sp = ctx.enter_context(tc.tile_pool(name="small", bufs=4))
        consts = ctx.enter_context(tc.tile_pool(name="consts", bufs=1))
        psum = ctx.enter_context(
            tc.tile_pool(name="psum", bufs=4, space=bass.MemorySpace.PSUM)
        )
        idt = consts.tile([PB, PB], BF16, tag="idt")
        nc.sync.dma_start(idt[:], idbf[:])

        def body():
            for mi, (src, dst) in enumerate(((us, unT), (vs, vnT))):
                # one batched DMA per shard: [1024, 256] -> [128, 8, 256]
                # u on the sync queue, v on the scalar queue (both HWDGE)
                dq = nc.sync if mi == 0 else nc.scalar
                x = pool.tile([PB, MB, D], F32, tag=f"x{mi}")
                dq.dma_start(x[:], src[:].rearrange("(t p) d -> p t d", p=PB))
                # squared row norms for all 8 blocks -> ss_all [128, 8]
                sq = pool.tile([PB, MB, D], F32, tag=f"sq{mi}")
                nc.vector.tensor_mul(sq[:], x[:], x[:])
                ss_all = sp.tile([PB, MB], F32, tag=f"ss{mi}")
                nc.vector.reduce_sum(ss_all[:], sq[:],
                                     axis=mybir.AxisListType.X)
                # r = 1/sqrt(ss), one batched Newton chain on [128, 8]
                nrm = sp.tile([PB, MB], F32, tag=f"nrm{mi}")
                nc.scalar.activation(nrm[:], ss_all[:], ACT.Sqrt)
                r = sp.tile([PB, MB], F32, tag=f"r0{mi}")
                nc.vector.reciprocal(r[:], nrm[:])
                for it in range(1):
                    t1 = sp.tile([PB, MB], F32, tag=f"t1_{it}{mi}")
                    nc.vector.tensor_mul(t1[:], r[:], r[:])
                    nc.vector.tensor_mul(t1[:], t1[:], ss_all[:])
                    nc.vector.tensor_scalar(t1[:], t1[:], -0.5, 1.5,
                                            ALU.mult, ALU.add)
                    rn = sp.tile([PB, MB], F32, tag=f"r{it + 1}{mi}")
                    nc.vector.tensor_mul(rn[:], r[:], t1[:])
                    r = rn
                y = pool.tile([PB, MB, D], BF16, tag=f"y{mi}")
                for t in range(MB):
                    nc.vector.tensor_scalar_mul(y[:, t, :], x[:, t, :],
                                                r[:, t:t + 1])
                # transpose 128x128 blocks on the (otherwise idle) PE
                yT = pool.tile([PB, KD, SH], BF16, tag=f"yT{mi}")
                for k in range(KD):
                    for t in range(MB):
                        pst = psum.tile([PB, PB], BF16, tag=f"pst{mi}")
                        nc.tensor.transpose(
                            pst[:], y[:, t, k * PB:(k + 1) * PB], idt[:]
                        )
                        nc.vector.tensor_copy(
                            yT[:, k, t * PB:(t + 1) * PB], pst[:]
                        )
                dq.dma_start(dst[:].rearrange("(k dd) c -> dd k c", dd=PB),
                             yT[:])

        if loop_n:
            with tc.For_i(0, loop_n, 1):
                body()
        else:
            for _rep in range(repeat):
                body()
    nc.compile()
    return nc


def _build_phase2(repeat=1, loop_n=0):
    """Similarity slab + streamed masked logsumexp.

    Inputs (per core, column-rolled so own rows are columns [0, 1024)):
      ur, vr [8192, 256] bf16; idmask/idneg [128, 128] f32 (eye, -300*eye).
    Output: loss [128, 8] f32; loss[p, m] is the per-row loss of local row
      m*128 + p.
    """
    nc = bacc.Bacc("TRN2", target_bir_lowering=False, debug=False)
    urT = nc.dram_tensor("urT", [D, B], BF16, kind="ExternalInput")
    vrT = nc.dram_tensor("vrT", [D, B], BF16, kind="ExternalInput")
    idmask = nc.dram_tensor("idmask", [PB, PB], F32, kind="ExternalInput")
    loss = nc.dram_tensor("loss", [PB, MB], F32, kind="ExternalOutput")

    with tile.TileContext(nc) as tc, ExitStack() as ctx:
        consts = ctx.enter_context(tc.tile_pool(name="consts", bufs=1))
        big = ctx.enter_context(tc.tile_pool(name="big", bufs=1))
        esc = ctx.enter_context(tc.tile_pool(name="esc", bufs=2))
        gs = ctx.enter_context(tc.tile_pool(name="gs", bufs=2))
        sm = ctx.enter_context(tc.tile_pool(name="sm", bufs=4))
        psum = ctx.enter_context(
            tc.tile_pool(name="psum", bufs=2, space=bass.MemorySpace.PSUM)
        )

        # constants
        idm = consts.tile([PB, PB], F32, tag="idm")
        nc.sync.dma_start(idm[:], idmask[:])
        biasc = consts.tile([PB, 1], F32, tag="biasc")
        nc.gpsimd.memset(biasc[:], -C)
        # trigger the exp/ln ACT table load early (overlaps input DMA)
        actwarm = consts.tile([PB, 1], F32, tag="actwarm")
        nc.scalar.activation(actwarm[:], biasc[:], ACT.Exp,
                             bias=biasc[:], scale=C)

        def body():
            # transposed unit matrices in 2048-column segments, split across
            # both HWDGE queues; matmuls start when the first segments land
            xT = {}
            qi = 0
            for s in range(NG):
                for nm, src in (("u", urT), ("v", vrT)):
                    for k in range(KD):
                        t = big.tile([PB, GROUP], BF16, tag=f"{nm}T{k}s{s}")
                        eng = nc.sync if qi % 2 == 0 else nc.gpsimd
                        eng.dma_start(
                            t[:],
                            src[k * PB:(k + 1) * PB,
                                s * GROUP:(s + 1) * GROUP],
                        )
                        qi += 1
                        xT[(nm, k, s)] = t

            # warm the PE (HAM clock gate) with throwaway matmuls on the
            # identity tile while the input DMAs stream in
            wps = psum.tile([PB, GROUP], F32, tag="ps")
            for _w in range(16):
                nc.tensor.matmul(wps[:, 0:PB], idm[:], idm[:],
                                 start=True, stop=True, skip_group_check=True)

            negsum_all = consts.tile([PB, MB], F32, tag="negsum_all")
            duv_all = consts.tile([PB, MB], F32, tag="duv_all")
            duu_all = consts.tile([PB, MB], F32, tag="duu_all")

            for m in range(MB):
                grpsum = gs.tile([PB, 2 * NG], F32, tag="grpsum")
                off = m * PB  # static diag offset in group 0 (rolled layout)
                for xi, nm in enumerate(("v", "u")):
                    for g in range(NG):
                        ps = psum.tile([PB, GROUP], F32, tag="ps")
                        # k outer: stationary operand stays loaded across the
                        # group (2 weight loads per 8 matmuls instead of 8)
                        for k in range(KD):
                            for q in range(NQ):
                                nc.tensor.matmul(
                                    ps[:, q * CHUNK:(q + 1) * CHUNK],
                                    xT[("u", k, 0)][:, m * PB:(m + 1) * PB],
                                    xT[(nm, k, g)][:, q * CHUNK:(q + 1) * CHUNK],
                                    start=(k == 0),
                                    stop=(k == KD - 1),
                                    skip_group_check=True,
                                )
                        if g == 0:
                            # read-only diagonal extraction (does not block
                            # the exp pass; its contribution is subtracted
                            # from the accumulated sums at the end)
                            dall = duv_all if nm == "v" else duu_all
                            scr = sm.tile([PB, PB], F32, tag="scr")
                            nc.vector.tensor_mul(
                                scr[:], ps[:, off:off + PB], idm[:]
                            )
                            nc.vector.reduce_sum(
                                dall[:, m:m + 1], scr[:],
                                axis=mybir.AxisListType.X,
                            )
                        escr = esc.tile([PB, GROUP], BF16, tag="escr")
                        col = xi * NG + g
                        nc.scalar.activation(
                            escr[:], ps[:], ACT.Exp,
                            bias=biasc[:], scale=C,
                            accum_out=grpsum[:, col:col + 1],
                        )
                nc.vector.reduce_sum(
                    negsum_all[:, m:m + 1], grpsum[:], axis=mybir.AxisListType.X
                )

            # tail (one Exp->Ln table-set switch per kernel):
            #   negsum -= exp((d_uv-1)C) + exp((d_uu-1)C)   [diag removal]
            #   loss = log(negsum) - (d_uv - 1)*C
            tuv = consts.tile([PB, MB], F32, tag="tuv")
            nc.scalar.activation(tuv[:], duv_all[:], ACT.Exp,
                                 bias=biasc[:], scale=C)
            tuu = consts.tile([PB, MB], F32, tag="tuu")
            nc.scalar.activation(tuu[:], duu_all[:], ACT.Exp,
                                 bias=biasc[:], scale=C)
            nc.vector.tensor_sub(negsum_all[:], negsum_all[:], tuv[:])
            nc.vector.tensor_sub(negsum_all[:], negsum_all[:], tuu[:])
            lg_all = consts.tile([PB, MB], F32, tag="lg_all")
            nc.scalar.activation(lg_all[:], negsum_all[:], ACT.Ln)
            posr_all = consts.tile([PB, MB], F32, tag="posr_all")
            nc.vector.tensor_scalar(posr_all[:], duv_all[:], -C, C,
                                    ALU.mult, ALU.add)
            lossT = consts.tile([PB, MB], F32, tag="loss")
            nc.vector.tensor_add(lossT[:], lg_all[:], posr_all[:])
            nc.sync.dma_start(loss[:], lossT[:])

        if loop_n:
            with tc.For_i(0, loop_n, 1):
                body()
        else:
            for _rep in range(repeat):
                body()
    nc.compile()
    return nc


def _get_programs():
    if "p1" not in _PROGRAMS:
        _PROGRAMS["p1"] = _build_phase1()
        _PROGRAMS["p2"] = _build_phase2()
    return _PROGRAMS["p1"], _PROGRAMS["p2"]


def make_phase1_inputs(u, v):
    import ml_dtypes
    eye_bf = np.eye(PB, dtype=ml_dtypes.bfloat16)
    return [
        {"us": u[c * SH:(c + 1) * SH], "vs": v[c * SH:(c + 1) * SH],
         "idbf": eye_bf}
        for c in range(NCORES)
    ]


def make_phase2_inputs(unT, vnT):
    """Per-core phase-2 input maps from the 8 normalized transposed shards,
    column-rolled so each core's own rows come first."""
    eye = np.eye(PB, dtype=np.float32)
    in2 = []
    for c in range(NCORES):
        in2.append({
            "urT": np.concatenate(unT[c:] + unT[:c], axis=1),
            "vrT": np.concatenate(vnT[c:] + vnT[:c], axis=1),
            "idmask": eye,
        })
    return in2


def run_phases(u, v):
    """Returns (loss_scalar, phase1_results, phase2_results)."""
    u = np.ascontiguousarray(np.asarray(u, dtype=np.float32))
    v = np.ascontiguousarray(np.asarray(v, dtype=np.float32))
    assert u.shape == (B, D) and v.shape == (B, D)
    p1, p2 = _get_programs()
    cores = list(range(NCORES))

    in1 = make_phase1_inputs(u, v)
    r1 = run_bass_kernel_spmd(p1, in1, cores)
    unT = [r1.results[c]["unT"] for c in cores]
    vnT = [r1.results[c]["vnT"] for c in cores]

    in2 = make_phase2_inputs(unT, vnT)
    r2 = run_bass_kernel_spmd(p2, in2, cores)
    losses = np.stack([r2.results[c]["loss"] for c in cores])  # [8, 128, 8]
    total = np.asarray(losses, dtype=np.float64).mean()
    return np.float32(total), r1, r2


def kernel(u, v):
    out, _, _ = run_phases(u, v)
    return np.asarray(out, dtype=np.float32)


if __name__ == "__main__":
    rng = np.random.default_rng(0)
    u = rng.standard_normal((B, D), dtype=np.float32)
    v = rng.standard_normal((B, D), dtype=np.float32)
    print("loss:", kernel(u, v))
